# revision 11
# baseline (speedup 1.0000x reference)
"""Trainium2 Bass kernel: LSTM encoder-decoder (IoT anomaly detector).

Reference semantics (B=256, T=512, I=128, H=256):
  encoder LSTM over x[B,T,I] -> final (h,c); pred_last = sigmoid(h @ lin_W.T + lin_b)
  decoder LSTM run T-1 steps feeding back its own prediction; outputs in
  forward time order [B,T,I].

Sharding: pure data parallelism, batch 256 -> 8 cores x 32.

Per-core layout (transposed): gate/hidden dims on SBUF partitions, batch on
the free dim.  Gate chunk order [g0 g1 f0 f1 i0 i1 o0 o1] so one Tanh covers
g and one Sigmoid covers f,i,o.  The per-core batch of 32 is split into S
phase-offset streams of C=32/S columns each; each stream runs its own
recurrence and the engines interleave streams to hide the serial chain
latency (PE -> ACT -> DVE -> ACT -> DVE per step).  State tiles (h, c) are
stream-major [s][k][C] so every per-stream op is one contiguous 2C-wide
instruction.

Gates PSUM tile per stream: one bank slice [gates 8*C | lin C].  All 8 chunk
biases enter via ONE matmul: lhsT rows 2m/2m+1 hold hi/lo bf16 halves of
chunk m's bias, rhs E has E[2m, Cm:Cm+C] = E[2m+1, ...] = 1.  h/pred feed
back as bf16; c stays f32; preds stored bf16 (DMA out bf16, host converts).
"""

import numpy as np
import ml_dtypes

B, T, I, H = 256, 512, 128, 256
NCORES = 8
LB = B // NCORES  # 32 local batch

BF16 = ml_dtypes.bfloat16

NSTREAMS = 2  # phase-offset streams (same for encoder and decoder)

_BUILT = {}


def _build(t_steps, S=NSTREAMS):
    import concourse.bass as bass
    import concourse.tile as tile
    from concourse import bacc, mybir

    f32 = mybir.dt.float32
    bf16 = mybir.dt.bfloat16
    AF = mybir.ActivationFunctionType
    ALU = mybir.AluOpType

    C = LB // S

    nc = bacc.Bacc(
        "TRN2", target_bir_lowering=False, debug=False, num_devices=NCORES
    )

    xT_d = nc.dram_tensor("xT", [128, t_steps * LB], bf16, kind="ExternalInput")
    wih_e_d = nc.dram_tensor("wih_e", [128, 8 * 128], bf16, kind="ExternalInput")
    whh_e_d = nc.dram_tensor("whh_e", [128, 16 * 128], bf16, kind="ExternalInput")
    bt_e_d = nc.dram_tensor("bt_e", [128, 128], bf16, kind="ExternalInput")
    wih_d_d = nc.dram_tensor("wih_d", [128, 8 * 128], bf16, kind="ExternalInput")
    whh_d_d = nc.dram_tensor("whh_d", [128, 16 * 128], bf16, kind="ExternalInput")
    bt_d_d = nc.dram_tensor("bt_d", [128, 128], bf16, kind="ExternalInput")
    wlin_d = nc.dram_tensor("wlin", [128, 2 * 128], bf16, kind="ExternalInput")
    bt_l_d = nc.dram_tensor("bt_l", [128, 128], bf16, kind="ExternalInput")
    eC_d = nc.dram_tensor("eC", [128, 8 * C], bf16, kind="ExternalInput")
    e0_d = nc.dram_tensor("e0", [128, LB], bf16, kind="ExternalInput")
    out_d = nc.dram_tensor("out", [128, t_steps * LB], bf16, kind="ExternalOutput")

    with tile.TileContext(nc) as tc:
        from contextlib import ExitStack

        with ExitStack() as ctx:
            const = ctx.enter_context(tc.tile_pool(name="const", bufs=1))
            work = ctx.enter_context(tc.tile_pool(name="work", bufs=2))
            psum = ctx.enter_context(
                tc.tile_pool(name="psum", bufs=2, space="PSUM")
            )

            def load(dram, shape, dt):
                t = const.tile(shape, dt, tag=dram.name)
                nc.sync.dma_start(out=t[:], in_=dram[:])
                return t

            xT = load(xT_d, [128, t_steps * LB], bf16)
            wih_e = load(wih_e_d, [128, 8 * 128], bf16)
            whh_e = load(whh_e_d, [128, 16 * 128], bf16)
            bt_e = load(bt_e_d, [128, 128], bf16)
            wih_dd = load(wih_d_d, [128, 8 * 128], bf16)
            whh_dd = load(whh_d_d, [128, 16 * 128], bf16)
            bt_dd = load(bt_d_d, [128, 128], bf16)
            wlin = load(wlin_d, [128, 2 * 128], bf16)
            bt_l = load(bt_l_d, [128, 128], bf16)
            eC = load(eC_d, [128, 8 * C], bf16)
            e0 = load(e0_d, [128, LB], bf16)

            preds = const.tile([128, t_steps * LB], bf16, tag="preds")
            # stream-major state: stream s k-tile k at cols 2C*s + C*k
            c_st = const.tile([128, 2 * LB], f32, tag="c_st")
            h_st = const.tile([128, 2 * LB], bf16, tag="h_st")

            nc.vector.memset(c_st[:], 0.0)
            nc.vector.memset(h_st[:], 0.0)

            # per-(stream, step) psum tiles, rotated via the pool (bufs=2)
            P_cur = [None] * S

            def stage_hh(s, dec):
                """bias + whh matmuls (need only h and constants)."""
                whh = whh_dd if dec else whh_e
                bt = bt_dd if dec else bt_e
                so = 2 * C * s
                hs = h_st[:, so : so + 2 * C]
                P = psum.tile([128, 9 * C], f32, tag=f"ps{s}", name=f"ps{s}")
                P_cur[s] = P
                nc.tensor.matmul(
                    P[:, 0 : 8 * C], bt[:], eC[:], start=True, stop=False,
                    skip_group_check=True,
                )
                for m in range(8):
                    for k in range(2):
                        nc.tensor.matmul(
                            P[:, C * m : C * m + C],
                            whh[:, 128 * (2 * m + k) : 128 * (2 * m + k + 1)],
                            hs[:, C * k : C * k + C],
                            start=False, stop=False, skip_group_check=True,
                        )

            def stage_ih(s, tau, dec, src_slot):
                """wih matmuls: enc reads x, dec reads the fed-back pred."""
                co = s * C
                wih = wih_dd if dec else wih_e
                P = P_cur[s]
                if dec:
                    rhs_x = preds[:, LB * src_slot + co : LB * src_slot + co + C]
                else:
                    rhs_x = xT[:, LB * tau + co : LB * tau + co + C]
                for m in range(8):
                    nc.tensor.matmul(
                        P[:, C * m : C * m + C],
                        wih[:, 128 * m : 128 * (m + 1)],
                        rhs_x,
                        start=False, stop=True, skip_group_check=True,
                    )

            def stage_sig(s):
                """gate activations: tanh over g chunks, sigmoid over f,i,o."""
                P = P_cur[s]
                Stl = work.tile([128, 8 * C], f32, tag=f"S{s}", name=f"S{s}")
                nc.scalar.activation(Stl[:, 0 : 2 * C], P[:, 0 : 2 * C], AF.Tanh)
                nc.scalar.activation(
                    Stl[:, 2 * C : 8 * C], P[:, 2 * C : 8 * C], AF.Sigmoid
                )
                return Stl

            def stage_tail(s, tau, Stl, emit_lin):
                """DVE cell math + tanh(c) + h (+ lin/pred)."""
                co = s * C
                so = 2 * C * s
                hs = h_st[:, so : so + 2 * C]
                cs = c_st[:, so : so + 2 * C]
                P = P_cur[s]
                G = Stl[:, 0 : 2 * C]
                Sf = Stl[:, 2 * C : 4 * C]
                Si = Stl[:, 4 * C : 6 * C]
                So = Stl[:, 6 * C : 8 * C]
                FU = work.tile([128, 4 * C], f32, tag=f"FU{s}", name=f"FU{s}")
                fc = FU[:, 0 : 2 * C]
                u = FU[:, 2 * C : 4 * C]
                nc.vector.tensor_tensor(fc, Sf, cs, ALU.mult)
                nc.vector.tensor_tensor(u, Si, G, ALU.mult)
                nc.vector.tensor_tensor(cs, fc, u, ALU.add)
                TC = work.tile([128, 2 * C], f32, tag=f"TC{s}", name=f"TC{s}")
                nc.scalar.activation(TC[:], cs, AF.Tanh)
                nc.vector.tensor_tensor(hs, So, TC[:], ALU.mult)
                if emit_lin:
                    lp = P[:, 8 * C : 9 * C]
                    nc.tensor.matmul(
                        lp, bt_l[:], e0[:, 0:C], start=True, stop=False,
                        skip_group_check=True,
                    )
                    for k in range(2):
                        nc.tensor.matmul(
                            lp, wlin[:, 128 * k : 128 * (k + 1)],
                            hs[:, C * k : C * k + C],
                            start=False, stop=(k == 1), skip_group_check=True,
                        )
                    nc.scalar.activation(
                        preds[:, LB * tau + co : LB * tau + co + C],
                        P[:, 8 * C : 9 * C], AF.Sigmoid,
                    )

            def step(tau, dec, emit_lin, src_slot):
                # PE: all streams' bias+whh first; decoder wih after (it
                # waits on sigmoid(pred) so it must not block other streams)
                for s in range(S):
                    stage_hh(s, dec)
                    if not dec:
                        stage_ih(s, tau, dec, src_slot)
                if dec:
                    for s in range(S):
                        stage_ih(s, tau, dec, src_slot)
                # ACT: all streams' gate sigmoids before any stream's tail,
                # so stream B's sigmoids run while stream A's DVE chain runs
                sigs = [stage_sig(s) for s in range(S)]
                for s in range(S):
                    stage_tail(s, tau, sigs[s], emit_lin)

            # encoder
            for tau in range(t_steps):
                step(tau, dec=False, emit_lin=(tau == t_steps - 1), src_slot=-1)
            # decoder: step k writes slot t_steps-2-k, reads slot t_steps-1-k
            for k in range(t_steps - 1):
                step(t_steps - 2 - k, dec=True, emit_lin=True,
                     src_slot=t_steps - 1 - k)

            nc.sync.dma_start(out=out_d[:], in_=preds[:])

    nc.compile()
    return nc


def _get(t_steps):
    if t_steps not in _BUILT:
        _BUILT[t_steps] = _build(t_steps)
    return _BUILT[t_steps]


# chunk order [g0 g1 f0 f1 i0 i1 o0 o1]; torch gate rows are [i f g o]
_CHUNKS = [4, 5, 2, 3, 0, 1, 6, 7]  # chunk -> torch 128-row block index


def _pack_weights(enc_W_ih, enc_W_hh, enc_b_ih, enc_b_hh,
                  dec_W_ih, dec_W_hh, dec_b_ih, dec_b_hh, lin_W, lin_b):
    def pack_ih(W):  # [4H, I] -> [128, 8*128] lhsT tiles, chunk-major
        return np.concatenate(
            [W[128 * cb : 128 * (cb + 1)].T for cb in _CHUNKS], axis=1
        ).astype(BF16)

    def pack_hh(W):  # [4H, H] -> [128, 16*128]: chunk m, k at col 128*(2m+k)
        tiles = []
        for cb in _CHUNKS:
            for k in range(2):
                tiles.append(
                    W[128 * cb : 128 * (cb + 1), 128 * k : 128 * (k + 1)].T
                )
        return np.concatenate(tiles, axis=1).astype(BF16)

    def pack_bias_T(b):  # [4H] -> [128, 128] lhsT: rows 2m/2m+1 = hi/lo
        out = np.zeros((128, 128), np.float32)
        for m, cb in enumerate(_CHUNKS):
            chunk = b[128 * cb : 128 * (cb + 1)].astype(np.float32)
            hi = chunk.astype(BF16).astype(np.float32)
            out[2 * m, :] = hi
            out[2 * m + 1, :] = chunk - hi
        return out.astype(BF16)

    C = LB // NSTREAMS
    e = np.zeros((128, 8 * C), np.float32)
    for m in range(8):
        e[2 * m, C * m : C * (m + 1)] = 1.0
        e[2 * m + 1, C * m : C * (m + 1)] = 1.0

    wlin = np.concatenate(
        [lin_W[:, 0:128].T, lin_W[:, 128:256].T], axis=1
    ).astype(BF16)
    btl = np.zeros((128, 128), np.float32)
    hi = lin_b.astype(BF16).astype(np.float32)
    btl[0, :] = hi
    btl[1, :] = lin_b - hi
    e0 = np.zeros((128, LB), np.float32)
    e0[0, :] = 1.0
    e0[1, :] = 1.0

    return {
        "wih_e": pack_ih(enc_W_ih),
        "whh_e": pack_hh(enc_W_hh),
        "bt_e": pack_bias_T(enc_b_ih + enc_b_hh),
        "wih_d": pack_ih(dec_W_ih),
        "whh_d": pack_hh(dec_W_hh),
        "bt_d": pack_bias_T(dec_b_ih + dec_b_hh),
        "wlin": wlin,
        "bt_l": btl.astype(BF16),
        "eC": e.astype(BF16),
        "e0": e0.astype(BF16),
    }


def _run(inputs, t_steps, trace=False):
    from concourse.bass_utils import run_bass_kernel_spmd

    nc = _get(t_steps)
    x = np.asarray(inputs["x"], np.float32)
    shared = _pack_weights(
        np.asarray(inputs["enc_W_ih"], np.float32),
        np.asarray(inputs["enc_W_hh"], np.float32),
        np.asarray(inputs["enc_b_ih"], np.float32),
        np.asarray(inputs["enc_b_hh"], np.float32),
        np.asarray(inputs["dec_W_ih"], np.float32),
        np.asarray(inputs["dec_W_hh"], np.float32),
        np.asarray(inputs["dec_b_ih"], np.float32),
        np.asarray(inputs["dec_b_hh"], np.float32),
        np.asarray(inputs["lin_W"], np.float32),
        np.asarray(inputs["lin_b"], np.float32),
    )
    in_maps = []
    for j in range(NCORES):
        xs = x[LB * j : LB * (j + 1), :t_steps]  # [32, T, 128]
        xT = np.ascontiguousarray(xs.transpose(2, 1, 0)).reshape(
            128, t_steps * LB
        )
        m = dict(shared)
        m["xT"] = xT.astype(BF16)
        in_maps.append(m)

    res = run_bass_kernel_spmd(nc, in_maps, list(range(NCORES)), trace=trace)
    out = np.empty((B, t_steps, I), np.float32)
    for j in range(NCORES):
        o = res.results[j]["out"].astype(np.float32).reshape(128, t_steps, LB)
        out[LB * j : LB * (j + 1)] = o.transpose(2, 1, 0)
    return out, res


def kernel(**inputs):
    out, _ = _run(inputs, T)
    return out
